# revision 8
# baseline (speedup 1.0000x reference)
"""Chamfer loss kernel for Trainium2 (8 NeuronCores, data-parallel over batch).

Problem: B=8, N=M=4096, D=3 fp32 point clouds.
  loss = mean_b mean_n min_m ||p_bn - g_bm||^2  +  mean_b mean_m min_n ||.||^2
  (squared euclidean, clamped at 0, matching pytorch3d norm=2 semantics)

Strategy (one batch element per core):
  - Distance tile d[n, m] = ||p||^2 + ||g||^2 - 2 p.g is produced by a K=5
    fp32 matmul on the PE: lhsT rows [1, ||p||^2, -2p0, -2p1, -2p2] against
    rhs rows [||g||^2, 1, g0, g1, g2]. 32 n-tiles x 8 m-tiles of [128, 512],
    written to PSUM in 4-bank spans of [128, 2048].
  - Row minima: fused vector.tensor_scalar reading each PSUM span once:
    out = relu(d) cast to bf16 in SBUF (feeds the column path), accum_out =
    running min along the free axis chained across the two spans per n-tile.
  - Column minima: bf16 tensor_tensor min accumulation into colacc[128, 4096]
    (elementwise across the 32 n-tiles), then a PE-transpose +
    free-axis-reduce tail to min over the partition axis.
  - Per-core scalar output (cham_x_b + cham_y_b); the host averages the 8
    per-core scalars (the data-parallel gather).

All arithmetic happens on-chip; the host only reshapes/transposes inputs
(layout) and averages the per-core partial losses (unshard).
"""

import os
import sys

import numpy as np

sys.path.insert(0, "/opt/trn_rl_repo")

import bass_rust
import concourse.bass as bass
import concourse.mybir as mybir
from concourse.bass_utils import run_bass_kernel_spmd
from concourse.masks import make_identity
from concourse.tile import TileContext

B, N, M, D = 8, 4096, 4096, 3
NT = N // 128  # 32 n-tiles
F32 = mybir.dt.float32
BF16 = mybir.dt.bfloat16
BIG = 3.0e38

# ---------------------------------------------------------------------------
# walrus in this container rejects >1 sync-wait per instruction; spill the
# extras onto engine-matched NoOps placed immediately before the instruction.
_nop_counter = [0]


def _split_multi_waits(nc):
    for func in nc.m.functions:
        for bb in func.blocks:
            out = []
            dirty = False
            for inst in bb.instructions:
                si = inst.sync_info
                if si is not None and len(si.on_wait) > 1:
                    waits = list(si.on_wait)
                    for w in waits[:-1]:
                        _nop_counter[0] += 1
                        nop = mybir.InstNoOp(
                            name=f"I-waitsplit-{_nop_counter[0]}", ins=[], outs=[]
                        )
                        nop.engine = inst.engine
                        nop.sync_info = bass_rust.SyncInfo(on_wait=[w], on_update=[])
                        out.append(nop)
                    inst.sync_info = bass_rust.SyncInfo(
                        on_wait=[waits[-1]], on_update=list(si.on_update)
                    )
                    dirty = True
                out.append(inst)
            if dirty:
                bb.instructions = out
    return nc


# ---------------------------------------------------------------------------


def _build_prep_side(nc, tc, pool, zrows, w_dram, t_dram, ident, scale, norm_row, coord_rows):
    """Load one point-cloud side and build its K=5 operand rows.

    zrows: [5, 4096] SBUF tile. coord_rows (3 of the 5) come from t_dram
    ([3, 4096], host-transposed), scaled by `scale`. norm_row gets the
    per-point squared norms, computed from the wide layout w_dram
    ([128, 96], point i = 128*t + p lives at partition p, cols 3t..3t+2)
    and flattened into n-order via PE-transpose + SBUF-to-SBUF DMA.
    """
    # coords. Compute ops must start at a 32-aligned partition in this
    # toolchain, so scale in a partition-0 staging tile and DMA into place.
    c0 = coord_rows[0]
    if scale != 1.0:
        stage = pool.tile([3, zrows.shape[1]], F32, name=f"stage_{t_dram.name}")
        nc.sync.dma_start(out=stage, in_=t_dram.ap())
        nc.vector.tensor_scalar(
            out=stage, in0=stage, scalar1=scale, scalar2=None, op0=mybir.AluOpType.mult
        )
        nc.sync.dma_start(out=zrows[c0 : c0 + 3, :], in_=stage)
    else:
        nc.sync.dma_start(out=zrows[c0 : c0 + 3, :], in_=t_dram.ap())
    # norms via wide layout
    w = pool.tile([128, 96], F32, name=f"w_{t_dram.name}")
    nc.sync.dma_start(out=w, in_=w_dram.ap())
    wsq = pool.tile([128, 96], F32, name=f"wsq_{t_dram.name}")
    nc.vector.tensor_mul(wsq, w, w)
    norms = pool.tile([128, NT], F32, name=f"norms_{t_dram.name}")
    nc.vector.tensor_reduce(
        out=norms,
        in_=wsq.rearrange("p (t d) -> p t d", d=3),
        axis=mybir.AxisListType.X,
        op=mybir.AluOpType.add,
    )
    # transpose [128, 32] -> [32, 128] so a flatten DMA yields n-order
    with tc.tile_pool(name=f"pst_{t_dram.name}", bufs=1, space="PSUM") as psp:
        tn_ps = psp.tile([128, 128], F32, name=f"tn_ps_{t_dram.name}")
        nc.tensor.matmul(
            tn_ps[0:NT, :], norms, ident, is_transpose=True, start=True, stop=True
        )
        tn = pool.tile([NT, 128], F32, name=f"tn_{t_dram.name}")
        nc.scalar.copy(tn, tn_ps[0:NT, :])
    # flatten [32, 128] -> [1, 4096] (c = 128*t + p): a size-matched DMA
    # iterates the source AP (partition-major) into the flat destination.
    nc.sync.dma_start(out=zrows[norm_row : norm_row + 1, :], in_=tn)


def build_nc():
    nc = bass.Bass("TRN2")
    predT = nc.dram_tensor("predT", [3, N], F32, kind="ExternalInput")
    gtT = nc.dram_tensor("gtT", [3, M], F32, kind="ExternalInput")
    predW = nc.dram_tensor("predW", [128, 3 * NT], F32, kind="ExternalInput")
    gtW = nc.dram_tensor("gtW", [128, 3 * NT], F32, kind="ExternalInput")
    out_d = nc.dram_tensor("out", [1, 1], F32, kind="ExternalOutput")

    with TileContext(nc) as tc:
        with (
            tc.tile_pool(name="persist", bufs=1) as persist,
            tc.tile_pool(name="dsb", bufs=3) as dsbp,
        ):
            ident = persist.tile([128, 128], F32)
            make_identity(nc, ident)
            ident16 = persist.tile([128, 128], BF16)
            make_identity(nc, ident16)

            # K=5 operand strips
            zp = persist.tile([5, N], F32)  # rows: [1, |p|^2, -2p0, -2p1, -2p2]
            zg = persist.tile([5, M], F32)  # rows: [|g|^2, 1, g0, g1, g2]
            # ones rows: memset must start at partition 0 in this walrus, so
            # fill the whole 5-row strip and let later writes overwrite.
            nc.vector.memset(zp, 1.0)
            nc.vector.memset(zg, 1.0)
            _build_prep_side(nc, tc, persist, zp, predW, predT, ident, -2.0, 1, (2,))
            _build_prep_side(nc, tc, persist, zg, gtW, gtT, ident, 1.0, 0, (2,))

            rowmins = persist.tile([128, NT], F32)
            colacc = persist.tile([128, M], BF16)

            # ---- main loop: 32 n-tiles x 2 half-spans of [128, 2048] ----
            with tc.tile_pool(name="psum_main", bufs=2, space="PSUM") as psp:
                for t in range(NT):
                    lhsT = zp[:, 128 * t : 128 * (t + 1)]
                    for h in range(2):
                        ps = psp.tile([128, 2048], F32, name="ps_main", tag="ps_main")
                        for j in range(4):
                            col0 = 2048 * h + 512 * j
                            nc.tensor.matmul(
                                ps[:, 512 * j : 512 * (j + 1)],
                                lhsT,
                                zg[:, col0 : col0 + 512],
                                start=True,
                                stop=True,
                            )
                        if t == 0:
                            dst = colacc[:, 2048 * h : 2048 * (h + 1)]
                        else:
                            dst = dsbp.tile([128, 2048], BF16, name="dsb", tag="dsb")
                        # fused: dst = relu(ps) (bf16), rowmin accum chained
                        nc.vector.tensor_scalar(
                            out=dst,
                            in0=ps,
                            scalar1=0.0,
                            scalar2=(BIG if h == 0 else rowmins[:, t : t + 1]),
                            op0=mybir.AluOpType.max,
                            op1=mybir.AluOpType.min,
                            accum_out=rowmins[:, t : t + 1],
                        )
                        if t > 0:
                            nc.vector.tensor_tensor(
                                out=colacc[:, 2048 * h : 2048 * (h + 1)],
                                in0=dst,
                                in1=colacc[:, 2048 * h : 2048 * (h + 1)],
                                op=mybir.AluOpType.min,
                            )

            # ---- tail: min over partitions of colacc via PE transpose ----
            colmins = persist.tile([128, NT], F32)
            with tc.tile_pool(name="psum_tail", bufs=2, space="PSUM") as psp2:
                for g in range(8):  # 8 groups of 4 [128,128] blocks
                    pst = psp2.tile([128, 512], BF16, name="ps_tr", tag="ps_tr")
                    for k in range(4):
                        b = 4 * g + k
                        nc.tensor.matmul(
                            pst[:, 128 * k : 128 * (k + 1)],
                            colacc[:, 128 * b : 128 * (b + 1)],
                            ident16,
                            is_transpose=True,
                            start=True,
                            stop=True,
                        )
                    nc.vector.tensor_reduce(
                        out=colmins[:, 4 * g : 4 * (g + 1)],
                        in_=pst.rearrange("p (k i) -> p k i", i=128),
                        axis=mybir.AxisListType.X,
                        op=mybir.AluOpType.min,
                    )

                # ---- final sums ----
                rowsum = persist.tile([128, 1], F32)
                colsum = persist.tile([128, 1], F32)
                nc.vector.tensor_reduce(
                    out=rowsum,
                    in_=rowmins,
                    axis=mybir.AxisListType.X,
                    op=mybir.AluOpType.add,
                )
                nc.vector.tensor_reduce(
                    out=colsum,
                    in_=colmins,
                    axis=mybir.AxisListType.X,
                    op=mybir.AluOpType.add,
                )
                total = persist.tile([128, 1], F32)
                nc.vector.tensor_add(total, rowsum, colsum)
                ps_s = psp2.tile([1, 1], F32, name="ps_s", tag="ps_s")
                ones = nc.const_aps.tensor(1.0, (128, 1))
                nc.tensor.matmul(ps_s, ones, total, start=True, stop=True)
                res_sb = persist.tile([1, 1], F32)
                nc.scalar.mul(res_sb, ps_s, 1.0 / float(N))
                nc.sync.dma_start(out=out_d.ap(), in_=res_sb)

    _split_multi_waits(nc)
    return nc


_NC = None


def _get_nc():
    global _NC
    if _NC is None:
        _NC = build_nc()
    return _NC


def _ensure_ntff_hook():
    """Register the axon NTFF profiling hook if the container's antenv stub
    lacks axon_hooks (trace support; harmless to skip)."""
    import types

    try:
        import antenv
    except ImportError:
        return
    if not hasattr(antenv, "axon_hooks") or not hasattr(
        getattr(antenv, "axon_hooks", None), "get_axon_ntff_profile_hook"
    ):
        mod = types.ModuleType("antenv.axon_hooks")
        mod._h = None
        mod.set_axon_ntff_profile_hook = lambda h: setattr(mod, "_h", h)
        mod.get_axon_ntff_profile_hook = lambda: mod._h
        sys.modules["antenv.axon_hooks"] = mod
        antenv.axon_hooks = mod
    from antenv import axon_hooks

    if axon_hooks.get_axon_ntff_profile_hook() is None:
        try:
            from trn_agent_boot.trn_boot import _ntff_profile_via_ctypes

            hook = _ntff_profile_via_ctypes("/opt/axon/libaxon_pjrt.so")
            if hook is not None:
                axon_hooks.set_axon_ntff_profile_hook(hook)
        except Exception:
            pass


def kernel(pred_points: np.ndarray, gt_points: np.ndarray, _want_trace: bool = False):
    pred = np.ascontiguousarray(np.asarray(pred_points, dtype=np.float32))
    gt = np.ascontiguousarray(np.asarray(gt_points, dtype=np.float32))
    assert pred.shape == (B, N, D) and gt.shape == (B, M, D)

    in_maps = []
    for b in range(B):
        p, g = pred[b], gt[b]
        in_maps.append(
            {
                "predT": np.ascontiguousarray(p.T),
                "gtT": np.ascontiguousarray(g.T),
                "predW": np.ascontiguousarray(
                    p.reshape(NT, 128, 3).transpose(1, 0, 2).reshape(128, 3 * NT)
                ),
                "gtW": np.ascontiguousarray(
                    g.reshape(NT, 128, 3).transpose(1, 0, 2).reshape(128, 3 * NT)
                ),
            }
        )

    nc = _get_nc()
    if _want_trace:
        _ensure_ntff_hook()
    res = run_bass_kernel_spmd(nc, in_maps, core_ids=list(range(B)), trace=_want_trace)
    per_core = np.array([r["out"][0, 0] for r in res.results], dtype=np.float64)
    loss = np.float32(per_core.mean())
    if _want_trace:
        return loss, res
    return loss


# revision 10
# speedup vs baseline: 1.7274x; 1.7274x over previous
"""Chamfer loss kernel for Trainium2 (8 NeuronCores, data-parallel over batch).

Problem: B=8, N=M=4096, D=3 fp32 point clouds.
  loss = mean_b mean_n min_m ||p_bn - g_bm||^2  +  mean_b mean_m min_n ||.||^2
  (squared euclidean, clamped at 0, matching pytorch3d norm=2 semantics)

Strategy (one batch element per core):
  - Distance tiles d[n, m] = ||p||^2 + ||g||^2 - 2 p.g come from K=7 float32r
    matmuls on the PE. float32r keeps ~12 mantissa bits, so the points are
    rounded once (consistently) and the squared norms are computed FROM the
    rounded points, split into hi+lo f32r rows so the norm contribution keeps
    full fp32 precision. Row pairing (lhsT row k x rhs row k):
       k0: 1 * |g|^2_hi   k1: 1 * |g|^2_lo
       k2: |p|^2_hi * 1   k3: |p|^2_lo * 1
       k4-6: (-2 p_d) * g_d
    The resulting d error is only the point-rounding perturbation (~2e-4
    relative on coordinates), which moves the final loss by ~1e-5 relative.
  - The K=7 strips are replicated at PE row-group partition bases
    {0, 32, 64, 96} and 4 matmuls run CONCURRENTLY via tile_position row
    tiling (measured ~59us for all 256 [128, 512] tiles vs 456us for fp32).
  - Row minima: fused vector.tensor_scalar reads each 4-bank PSUM span once:
    out = relu(d) cast to bf16 into SBUF (feeds the column path), accum_out =
    running min along free, chained across the two spans per n-tile.
  - Column minima: bf16 tensor_tensor min into colacc[128, 4096], then a
    PE-transpose + free-axis reduce tail for the partition-axis min.
  - Per-core scalar output (cham_x_b + cham_y_b); the host averages the 8
    per-core scalars (the data-parallel gather).

All arithmetic happens on-chip; the host only reshapes/transposes inputs
(layout) and averages the per-core partial losses (unshard).
"""

import os
import sys

import numpy as np

sys.path.insert(0, "/opt/trn_rl_repo")

import bass_rust
import concourse.bass as bass
import concourse.mybir as mybir
from concourse.bass_utils import run_bass_kernel_spmd
from concourse.masks import make_identity
from concourse.tile import TileContext

B, N, M, D = 8, 4096, 4096, 3
NT = N // 128  # 32 n-tiles
K = 7
F32 = mybir.dt.float32
F32R = mybir.dt.float32r
BF16 = mybir.dt.bfloat16
BIG = 3.0e38

# ---------------------------------------------------------------------------
# walrus in this container rejects >1 sync-wait per instruction; spill the
# extras onto engine-matched NoOps placed immediately before the instruction.
_nop_counter = [0]


def _split_multi_waits(nc):
    for func in nc.m.functions:
        for bb in func.blocks:
            out = []
            dirty = False
            for inst in bb.instructions:
                si = inst.sync_info
                if si is not None and len(si.on_wait) > 1:
                    waits = list(si.on_wait)
                    for w in waits[:-1]:
                        _nop_counter[0] += 1
                        nop = mybir.InstNoOp(
                            name=f"I-waitsplit-{_nop_counter[0]}", ins=[], outs=[]
                        )
                        nop.engine = inst.engine
                        nop.sync_info = bass_rust.SyncInfo(on_wait=[w], on_update=[])
                        out.append(nop)
                    inst.sync_info = bass_rust.SyncInfo(
                        on_wait=[waits[-1]], on_update=list(si.on_update)
                    )
                    dirty = True
                out.append(inst)
            if dirty:
                bb.instructions = out
    return nc


# ---------------------------------------------------------------------------


def _build_prep_side(nc, tc, pool, zrows, w_dram, t_dram, ident, scale, hi_row, lo_row, c_row):
    """Build one side's K=7 rows (at partition base 0) in `zrows` [K, 4096] f32r.

    - rows c_row..c_row+2: f32r-rounded coords (scaled by `scale`), from the
      host-transposed [3, 4096] layout.
    - rows hi_row/lo_row: hi/lo split of the squared norms of the ROUNDED
      points, computed in the wide [128, 96] layout (point 128t+p at
      partition p, cols 3t..3t+2) and flattened into n-order via
      PE-transpose + a size-matched SBUF-to-SBUF DMA.
    Ones rows are pre-filled by the caller's memset.
    """
    nm = t_dram.name
    # --- coords: fp32 staging -> scale -> f32r (round) ---
    stage = pool.tile([3, N], F32, name=f"stage_{nm}")
    nc.sync.dma_start(out=stage, in_=t_dram.ap())
    coords_r = pool.tile([3, N], F32R, name=f"coords_{nm}")
    nc.vector.tensor_scalar(
        out=coords_r, in0=stage, scalar1=scale, scalar2=None, op0=mybir.AluOpType.mult
    )
    nc.sync.dma_start(out=zrows[c_row : c_row + 3, :], in_=coords_r)

    # --- norms of rounded points, wide layout ---
    w = pool.tile([128, 3 * NT], F32, name=f"w_{nm}")
    nc.sync.dma_start(out=w, in_=w_dram.ap())
    wr = pool.tile([128, 3 * NT], F32R, name=f"wr_{nm}")
    # match the coords rounding exactly: scale then round, then undo the
    # scale sign on squaring (scale^2 factored out below)
    nc.vector.tensor_scalar(
        out=wr, in0=w, scalar1=scale, scalar2=None, op0=mybir.AluOpType.mult
    )
    wsq = pool.tile([128, 3 * NT], F32, name=f"wsq_{nm}")
    # (scale*p)^2 = scale^2 * p^2 -> divide back by scale^2 via tensor_scalar
    nc.vector.tensor_mul(wsq, wr, wr)
    norms = pool.tile([128, NT], F32, name=f"norms_{nm}")
    nc.vector.tensor_reduce(
        out=norms,
        in_=wsq.rearrange("p (t d) -> p t d", d=3),
        axis=mybir.AxisListType.X,
        op=mybir.AluOpType.add,
    )
    if scale != 1.0:
        nc.vector.tensor_scalar(
            out=norms,
            in0=norms,
            scalar1=1.0 / (scale * scale),
            scalar2=None,
            op0=mybir.AluOpType.mult,
        )
    # hi/lo split
    nh = pool.tile([128, NT], F32R, name=f"nh_{nm}")
    nc.vector.tensor_copy(nh, norms)
    nl_f = pool.tile([128, NT], F32, name=f"nlf_{nm}")
    nc.vector.tensor_sub(nl_f, norms, nh.bitcast(F32))
    nl = pool.tile([128, NT], F32R, name=f"nl_{nm}")
    nc.vector.tensor_copy(nl, nl_f)
    # transpose [128, 32] -> [32, 128] and flatten (c = 128 t + p) into zrows
    with tc.tile_pool(name=f"pst_{nm}", bufs=1, space="PSUM") as psp:
        for row, src in ((hi_row, nh), (lo_row, nl)):
            tn_ps = psp.tile([128, 128], F32, name=f"tnps_{nm}_{row}", tag="tnps")
            nc.tensor.matmul(
                tn_ps[0:NT, :],
                src.bitcast(F32),
                ident,
                is_transpose=True,
                start=True,
                stop=True,
            )
            tn = pool.tile([NT, 128], F32R, name=f"tn_{nm}_{row}")
            nc.scalar.copy(tn, tn_ps[0:NT, :])
            nc.sync.dma_start(out=zrows[row : row + 1, :], in_=tn)


def build_nc():
    nc = bass.Bass("TRN2")
    predT = nc.dram_tensor("predT", [3, N], F32, kind="ExternalInput")
    gtT = nc.dram_tensor("gtT", [3, M], F32, kind="ExternalInput")
    predW = nc.dram_tensor("predW", [128, 3 * NT], F32, kind="ExternalInput")
    gtW = nc.dram_tensor("gtW", [128, 3 * NT], F32, kind="ExternalInput")
    out_d = nc.dram_tensor("out", [1, 1], F32, kind="ExternalOutput")

    with TileContext(nc) as tc:
        with (
            tc.tile_pool(name="persist", bufs=1) as persist,
            tc.tile_pool(name="dsb", bufs=3) as dsbp,
        ):
            ident = persist.tile([128, 128], F32)
            make_identity(nc, ident)
            ident16 = persist.tile([128, 128], BF16)
            make_identity(nc, ident16)

            # K=7 operand strips at partition base 0
            zp = persist.tile([K, N], F32R)
            zg = persist.tile([K, M], F32R)
            # ones rows: Memset can't target f32r, and compute ops can't
            # start at unaligned partitions — build a rounded ones strip at
            # partition 0 and DMA it into place.
            ones_f = persist.tile([2, N], F32)
            nc.vector.memset(ones_f, 1.0)
            ones_r = persist.tile([2, N], F32R)
            nc.vector.tensor_copy(ones_r, ones_f)
            nc.sync.dma_start(out=zp[0:2, :], in_=ones_r)
            nc.sync.dma_start(out=zg[2:4, :], in_=ones_r)
            # zp rows: [1, 1, nPh, nPl, -2p0, -2p1, -2p2]
            # zg rows: [nGh, nGl, 1, 1, g0, g1, g2]
            _build_prep_side(nc, tc, persist, zp, predW, predT, ident, -2.0, 2, 3, 4)
            _build_prep_side(nc, tc, persist, zg, gtW, gtT, ident, 1.0, 0, 1, 4)

            # replicate both strips at PE row-group bases 32/64/96
            zp_pk = persist.tile([96 + K, N], F32R)
            zg_pk = persist.tile([96 + K, M], F32R)
            for g in range(4):
                nc.sync.dma_start(out=zp_pk[32 * g : 32 * g + K, :], in_=zp)
                nc.sync.dma_start(out=zg_pk[32 * g : 32 * g + K, :], in_=zg)

            rowmins = persist.tile([128, NT], F32)
            colacc = persist.tile([128, M], BF16)

            # ---- main loop: 32 n-tiles x 2 half-spans of [128, 2048];
            #      each span = 4 row-group-packed concurrent matmuls ----
            with tc.tile_pool(name="psum_main", bufs=2, space="PSUM") as psp:
                for t in range(NT):
                    for h in range(2):
                        ps = psp.tile([128, 2048], F32, name="ps_main", tag="ps_main")
                        for g in range(4):
                            col0 = 2048 * h + 512 * g
                            nc.tensor.matmul(
                                ps[:, 512 * g : 512 * (g + 1)],
                                zp_pk[32 * g : 32 * g + K, 128 * t : 128 * (t + 1)],
                                zg_pk[32 * g : 32 * g + K, col0 : col0 + 512],
                                start=True,
                                stop=True,
                                tile_position=(32 * g, 0),
                            )
                        if t == 0:
                            dst = colacc[:, 2048 * h : 2048 * (h + 1)]
                        else:
                            dst = dsbp.tile([128, 2048], BF16, name="dsb", tag="dsb")
                        # fused: dst = relu(ps) (bf16), rowmin accum chained
                        nc.vector.tensor_scalar(
                            out=dst,
                            in0=ps,
                            scalar1=0.0,
                            scalar2=(BIG if h == 0 else rowmins[:, t : t + 1]),
                            op0=mybir.AluOpType.max,
                            op1=mybir.AluOpType.min,
                            accum_out=rowmins[:, t : t + 1],
                        )
                        if t > 0:
                            nc.vector.tensor_tensor(
                                out=colacc[:, 2048 * h : 2048 * (h + 1)],
                                in0=dst,
                                in1=colacc[:, 2048 * h : 2048 * (h + 1)],
                                op=mybir.AluOpType.min,
                            )

            # ---- tail: min over partitions of colacc via PE transpose ----
            colmins = persist.tile([128, NT], F32)
            with tc.tile_pool(name="psum_tail", bufs=2, space="PSUM") as psp2:
                for g in range(8):  # 8 groups of 4 [128,128] blocks
                    pst = psp2.tile([128, 512], BF16, name="ps_tr", tag="ps_tr")
                    for k in range(4):
                        b = 4 * g + k
                        nc.tensor.matmul(
                            pst[:, 128 * k : 128 * (k + 1)],
                            colacc[:, 128 * b : 128 * (b + 1)],
                            ident16,
                            is_transpose=True,
                            start=True,
                            stop=True,
                        )
                    nc.vector.tensor_reduce(
                        out=colmins[:, 4 * g : 4 * (g + 1)],
                        in_=pst.rearrange("p (k i) -> p k i", i=128),
                        axis=mybir.AxisListType.X,
                        op=mybir.AluOpType.min,
                    )

                # ---- final sums ----
                rowsum = persist.tile([128, 1], F32)
                colsum = persist.tile([128, 1], F32)
                nc.vector.tensor_reduce(
                    out=rowsum,
                    in_=rowmins,
                    axis=mybir.AxisListType.X,
                    op=mybir.AluOpType.add,
                )
                nc.vector.tensor_reduce(
                    out=colsum,
                    in_=colmins,
                    axis=mybir.AxisListType.X,
                    op=mybir.AluOpType.add,
                )
                total = persist.tile([128, 1], F32)
                nc.vector.tensor_add(total, rowsum, colsum)
                ps_s = psp2.tile([1, 1], F32, name="ps_s", tag="ps_s")
                ones = nc.const_aps.tensor(1.0, (128, 1))
                nc.tensor.matmul(ps_s, ones, total, start=True, stop=True)
                res_sb = persist.tile([1, 1], F32)
                nc.scalar.mul(res_sb, ps_s, 1.0 / float(N))
                nc.sync.dma_start(out=out_d.ap(), in_=res_sb)

    _split_multi_waits(nc)
    return nc


_NC = None


def _get_nc():
    global _NC
    if _NC is None:
        _NC = build_nc()
    return _NC


def _ensure_ntff_hook():
    """Register the axon NTFF profiling hook if the container's antenv stub
    lacks axon_hooks (trace support; harmless to skip)."""
    import types

    try:
        import antenv
    except ImportError:
        return
    if not hasattr(antenv, "axon_hooks") or not hasattr(
        getattr(antenv, "axon_hooks", None), "get_axon_ntff_profile_hook"
    ):
        mod = types.ModuleType("antenv.axon_hooks")
        mod._h = None
        mod.set_axon_ntff_profile_hook = lambda h: setattr(mod, "_h", h)
        mod.get_axon_ntff_profile_hook = lambda: mod._h
        sys.modules["antenv.axon_hooks"] = mod
        antenv.axon_hooks = mod
    from antenv import axon_hooks

    if axon_hooks.get_axon_ntff_profile_hook() is None:
        try:
            from trn_agent_boot.trn_boot import _ntff_profile_via_ctypes

            hook = _ntff_profile_via_ctypes("/opt/axon/libaxon_pjrt.so")
            if hook is not None:
                axon_hooks.set_axon_ntff_profile_hook(hook)
        except Exception:
            pass


def kernel(pred_points: np.ndarray, gt_points: np.ndarray, _want_trace: bool = False):
    pred = np.ascontiguousarray(np.asarray(pred_points, dtype=np.float32))
    gt = np.ascontiguousarray(np.asarray(gt_points, dtype=np.float32))
    assert pred.shape == (B, N, D) and gt.shape == (B, M, D)

    in_maps = []
    for b in range(B):
        p, g = pred[b], gt[b]
        in_maps.append(
            {
                "predT": np.ascontiguousarray(p.T),
                "gtT": np.ascontiguousarray(g.T),
                "predW": np.ascontiguousarray(
                    p.reshape(NT, 128, 3).transpose(1, 0, 2).reshape(128, 3 * NT)
                ),
                "gtW": np.ascontiguousarray(
                    g.reshape(NT, 128, 3).transpose(1, 0, 2).reshape(128, 3 * NT)
                ),
            }
        )

    nc = _get_nc()
    if _want_trace:
        _ensure_ntff_hook()
    res = run_bass_kernel_spmd(nc, in_maps, core_ids=list(range(B)), trace=_want_trace)
    per_core = np.array([r["out"][0, 0] for r in res.results], dtype=np.float64)
    loss = np.float32(per_core.mean())
    if _want_trace:
        return loss, res
    return loss


# revision 13
# speedup vs baseline: 1.7586x; 1.0180x over previous
"""Chamfer loss kernel for Trainium2 (8 NeuronCores, data-parallel over batch).

Problem: B=8, N=M=4096, D=3 fp32 point clouds.
  loss = mean_b mean_n min_m ||p_bn - g_bm||^2  +  mean_b mean_m min_n ||.||^2
  (squared euclidean, clamped at 0, matching pytorch3d norm=2 semantics)

Strategy (one batch element per core):
  - Distance tiles d[n, m] = ||p||^2 + ||g||^2 - 2 p.g come from K=7 float32r
    matmuls on the PE. float32r keeps ~12 mantissa bits, so the points are
    rounded once (consistently) and the squared norms are computed FROM the
    rounded points, split into hi+lo f32r rows so the norm contribution keeps
    full fp32 precision. Row pairing (lhsT row k x rhs row k):
       k0: 1 * |g|^2_hi   k1: 1 * |g|^2_lo
       k2: |p|^2_hi * 1   k3: |p|^2_lo * 1
       k4-6: (-2 p_d) * g_d
    The resulting d error is only the point-rounding perturbation (~2e-4
    relative on coordinates), which moves the final loss by ~1e-5 relative.
  - The K=7 strips are replicated at PE row-group partition bases
    {0, 32, 64, 96} and 4 matmuls run CONCURRENTLY via tile_position row
    tiling (measured ~59us for all 256 [128, 512] tiles vs 456us for fp32).
  - Row minima: fused vector.tensor_scalar reads each 4-bank PSUM span once:
    out = relu(d) cast to bf16 into SBUF (feeds the column path), accum_out =
    running min along free, chained across the two spans per n-tile.
  - Column minima: bf16 tensor_tensor min into colacc[128, 4096], then a
    PE-transpose + free-axis reduce tail for the partition-axis min.
  - Per-core scalar output (cham_x_b + cham_y_b); the host averages the 8
    per-core scalars (the data-parallel gather).

All arithmetic happens on-chip; the host only reshapes/transposes inputs
(layout) and averages the per-core partial losses (unshard).
"""

import os
import sys

import numpy as np

sys.path.insert(0, "/opt/trn_rl_repo")

import bass_rust
import concourse.bass as bass
import concourse.mybir as mybir
from concourse.bass_utils import run_bass_kernel_spmd
from concourse.masks import make_identity
from concourse.tile import TileContext

B, N, M, D = 8, 4096, 4096, 3
NT = N // 128  # 32 n-tiles
K = 7
F32 = mybir.dt.float32
F32R = mybir.dt.float32r
BF16 = mybir.dt.bfloat16
BIG = 3.0e38

# ---------------------------------------------------------------------------
# walrus in this container rejects >1 sync-wait per instruction; spill the
# extras onto engine-matched NoOps placed immediately before the instruction.
_nop_counter = [0]


def _split_multi_waits(nc):
    for func in nc.m.functions:
        for bb in func.blocks:
            out = []
            dirty = False
            for inst in bb.instructions:
                si = inst.sync_info
                if si is not None and len(si.on_wait) > 1:
                    waits = list(si.on_wait)
                    for w in waits[:-1]:
                        _nop_counter[0] += 1
                        nop = mybir.InstNoOp(
                            name=f"I-waitsplit-{_nop_counter[0]}", ins=[], outs=[]
                        )
                        nop.engine = inst.engine
                        nop.sync_info = bass_rust.SyncInfo(on_wait=[w], on_update=[])
                        out.append(nop)
                    inst.sync_info = bass_rust.SyncInfo(
                        on_wait=[waits[-1]], on_update=list(si.on_update)
                    )
                    dirty = True
                out.append(inst)
            if dirty:
                bb.instructions = out
    return nc


# ---------------------------------------------------------------------------


def _build_prep_side(nc, tc, pool, zrows, w_dram, t_dram, ident, scale, hi_row, lo_row, c_row):
    """Build one side's K=7 rows (at partition base 0) in `zrows` [K, 4096] f32r.

    - rows c_row..c_row+2: f32r-rounded coords (scaled by `scale`), from the
      host-transposed [3, 4096] layout.
    - rows hi_row/lo_row: hi/lo split of the squared norms of the ROUNDED
      points, computed in the wide [128, 96] layout (point 128t+p at
      partition p, cols 3t..3t+2) and flattened into n-order via
      PE-transpose + a size-matched SBUF-to-SBUF DMA.
    Ones rows are pre-filled by the caller's memset.
    """
    nm = t_dram.name
    # --- coords: fp32 staging -> scale -> f32r (round; on ACT, DVE is busy) ---
    stage = pool.tile([3, N], F32, name=f"stage_{nm}")
    nc.sync.dma_start(out=stage, in_=t_dram.ap())
    coords_r = pool.tile([3, N], F32R, name=f"coords_{nm}")
    nc.scalar.mul(out=coords_r, in_=stage, mul=scale)
    nc.scalar.dma_start(out=zrows[c_row : c_row + 3, :], in_=coords_r)

    # --- norms of rounded points, wide layout ---
    w = pool.tile([128, 3 * NT], F32, name=f"w_{nm}")
    nc.sync.dma_start(out=w, in_=w_dram.ap())
    wr = pool.tile([128, 3 * NT], F32R, name=f"wr_{nm}")
    # match the coords rounding exactly: scale then round, then undo the
    # scale sign on squaring (scale^2 factored out below)
    nc.scalar.mul(out=wr, in_=w, mul=scale)
    wsq = pool.tile([128, 3 * NT], F32, name=f"wsq_{nm}")
    # (scale*p)^2 = scale^2 * p^2 -> divide back by scale^2 via tensor_scalar
    nc.vector.tensor_mul(wsq, wr, wr)
    norms = pool.tile([128, NT], F32, name=f"norms_{nm}")
    nc.vector.tensor_reduce(
        out=norms,
        in_=wsq.rearrange("p (t d) -> p t d", d=3),
        axis=mybir.AxisListType.X,
        op=mybir.AluOpType.add,
    )
    if scale != 1.0:
        nc.vector.tensor_scalar(
            out=norms,
            in0=norms,
            scalar1=1.0 / (scale * scale),
            scalar2=None,
            op0=mybir.AluOpType.mult,
        )
    # hi/lo split
    nh = pool.tile([128, NT], F32R, name=f"nh_{nm}")
    nc.vector.tensor_copy(nh, norms)
    nl_f = pool.tile([128, NT], F32, name=f"nlf_{nm}")
    nc.vector.tensor_sub(nl_f, norms, nh.bitcast(F32))
    nl = pool.tile([128, NT], F32R, name=f"nl_{nm}")
    nc.vector.tensor_copy(nl, nl_f)
    # transpose [128, 32] -> [32, 128] and flatten (c = 128 t + p) into zrows
    with tc.tile_pool(name=f"pst_{nm}", bufs=1, space="PSUM") as psp:
        for row, src in ((hi_row, nh), (lo_row, nl)):
            tn_ps = psp.tile([128, 128], F32, name=f"tnps_{nm}_{row}", tag="tnps")
            nc.tensor.matmul(
                tn_ps[0:NT, :],
                src.bitcast(F32),
                ident,
                is_transpose=True,
                start=True,
                stop=True,
            )
            tn = pool.tile([NT, 128], F32R, name=f"tn_{nm}_{row}")
            nc.scalar.copy(tn, tn_ps[0:NT, :])
            nc.sync.dma_start(out=zrows[row : row + 1, :], in_=tn)


def build_nc():
    nc = bass.Bass("TRN2")
    predT = nc.dram_tensor("predT", [3, N], F32, kind="ExternalInput")
    gtT = nc.dram_tensor("gtT", [3, M], F32, kind="ExternalInput")
    predW = nc.dram_tensor("predW", [128, 3 * NT], F32, kind="ExternalInput")
    gtW = nc.dram_tensor("gtW", [128, 3 * NT], F32, kind="ExternalInput")
    out_d = nc.dram_tensor("out", [1, 1], F32, kind="ExternalOutput")

    with TileContext(nc) as tc:
        with (
            tc.tile_pool(name="persist", bufs=1) as persist,
            tc.tile_pool(name="dsb", bufs=3) as dsbp,
        ):
            ident = persist.tile([128, 128], F32)
            make_identity(nc, ident)
            ident16 = persist.tile([128, 128], BF16)
            make_identity(nc, ident16)

            # K=7 operand strips, built directly at row-group base 0 of the
            # packed tiles, then replicated at bases 32/64/96.
            zp_pk = persist.tile([96 + K, N], F32R)
            zg_pk = persist.tile([96 + K, M], F32R)
            zp = zp_pk[0:K, :]
            zg = zg_pk[0:K, :]
            # ones rows: Memset can't target f32r, and compute ops can't
            # start at unaligned partitions — build a rounded ones strip at
            # partition 0 and DMA it into place.
            ones_f = persist.tile([2, N], F32)
            nc.gpsimd.memset(ones_f, 1.0)
            ones_r = persist.tile([2, N], F32R)
            nc.scalar.copy(ones_r, ones_f)
            nc.sync.dma_start(out=zp[0:2, :], in_=ones_r)
            nc.scalar.dma_start(out=zg[2:4, :], in_=ones_r)
            # zp rows: [1, 1, nPh, nPl, -2p0, -2p1, -2p2]
            # zg rows: [nGh, nGl, 1, 1, g0, g1, g2]
            _build_prep_side(nc, tc, persist, zp, predW, predT, ident, -2.0, 2, 3, 4)
            _build_prep_side(nc, tc, persist, zg, gtW, gtT, ident, 1.0, 0, 1, 4)

            # replicate both strips at PE row-group bases 32/64/96,
            # split across the two HWDGE queues
            for g in range(1, 4):
                eng = nc.sync if g % 2 else nc.scalar
                eng.dma_start(out=zp_pk[32 * g : 32 * g + K, :], in_=zp)
                eng2 = nc.scalar if g % 2 else nc.sync
                eng2.dma_start(out=zg_pk[32 * g : 32 * g + K, :], in_=zg)

            rowmins = persist.tile([128, NT], F32)
            colacc = persist.tile([128, M], BF16)

            # ---- main loop: 32 n-tiles x 2 half-spans of [128, 2048];
            #      each span = 4 row-group-packed concurrent matmuls ----
            with tc.tile_pool(name="psum_main", bufs=2, space="PSUM") as psp:
                for t in range(NT):
                    dsb = (
                        None
                        if t == 0
                        else dsbp.tile([128, M], BF16, name="dsb", tag="dsb")
                    )
                    for h in range(2):
                        ps = psp.tile([128, 2048], F32, name="ps_main", tag="ps_main")
                        for g in range(4):
                            col0 = 2048 * h + 512 * g
                            nc.tensor.matmul(
                                ps[:, 512 * g : 512 * (g + 1)],
                                zp_pk[32 * g : 32 * g + K, 128 * t : 128 * (t + 1)],
                                zg_pk[32 * g : 32 * g + K, col0 : col0 + 512],
                                start=True,
                                stop=True,
                                tile_position=(32 * g, 0),
                            )
                        if t == 0:
                            dst = colacc[:, 2048 * h : 2048 * (h + 1)]
                        else:
                            dst = dsb[:, 2048 * h : 2048 * (h + 1)]
                        # fused: dst = relu(ps) (bf16), rowmin accum chained
                        nc.vector.tensor_scalar(
                            out=dst,
                            in0=ps,
                            scalar1=0.0,
                            scalar2=(BIG if h == 0 else rowmins[:, t : t + 1]),
                            op0=mybir.AluOpType.max,
                            op1=mybir.AluOpType.min,
                            accum_out=rowmins[:, t : t + 1],
                        )
                    if t > 0:
                        nc.vector.tensor_tensor(
                            out=colacc,
                            in0=dsb,
                            in1=colacc,
                            op=mybir.AluOpType.min,
                        )

            # ---- tail: min over partitions of colacc via PE transpose ----
            colmins = persist.tile([128, NT], F32)
            with tc.tile_pool(name="psum_tail", bufs=2, space="PSUM") as psp2:
                for g in range(8):  # 8 groups of 4 [128,128] blocks
                    pst = psp2.tile([128, 512], BF16, name="ps_tr", tag="ps_tr")
                    for k in range(4):
                        b = 4 * g + k
                        nc.tensor.matmul(
                            pst[:, 128 * k : 128 * (k + 1)],
                            colacc[:, 128 * b : 128 * (b + 1)],
                            ident16,
                            is_transpose=True,
                            start=True,
                            stop=True,
                        )
                    nc.vector.tensor_reduce(
                        out=colmins[:, 4 * g : 4 * (g + 1)],
                        in_=pst.rearrange("p (k i) -> p k i", i=128),
                        axis=mybir.AxisListType.X,
                        op=mybir.AluOpType.min,
                    )

                # ---- final sums ----
                rowsum = persist.tile([128, 1], F32)
                colsum = persist.tile([128, 1], F32)
                nc.vector.tensor_reduce(
                    out=rowsum,
                    in_=rowmins,
                    axis=mybir.AxisListType.X,
                    op=mybir.AluOpType.add,
                )
                nc.vector.tensor_reduce(
                    out=colsum,
                    in_=colmins,
                    axis=mybir.AxisListType.X,
                    op=mybir.AluOpType.add,
                )
                total = persist.tile([128, 1], F32)
                nc.vector.tensor_add(total, rowsum, colsum)
                ps_s = psp2.tile([1, 1], F32, name="ps_s", tag="ps_s")
                ones = nc.const_aps.tensor(1.0, (128, 1))
                nc.tensor.matmul(ps_s, ones, total, start=True, stop=True)
                res_sb = persist.tile([1, 1], F32)
                nc.scalar.mul(res_sb, ps_s, 1.0 / float(N))
                nc.sync.dma_start(out=out_d.ap(), in_=res_sb)

    _split_multi_waits(nc)
    return nc


_NC = None


def _get_nc():
    global _NC
    if _NC is None:
        _NC = build_nc()
    return _NC


def _ensure_ntff_hook():
    """Register the axon NTFF profiling hook if the container's antenv stub
    lacks axon_hooks (trace support; harmless to skip)."""
    import types

    try:
        import antenv
    except ImportError:
        return
    if not hasattr(antenv, "axon_hooks") or not hasattr(
        getattr(antenv, "axon_hooks", None), "get_axon_ntff_profile_hook"
    ):
        mod = types.ModuleType("antenv.axon_hooks")
        mod._h = None
        mod.set_axon_ntff_profile_hook = lambda h: setattr(mod, "_h", h)
        mod.get_axon_ntff_profile_hook = lambda: mod._h
        sys.modules["antenv.axon_hooks"] = mod
        antenv.axon_hooks = mod
    from antenv import axon_hooks

    if axon_hooks.get_axon_ntff_profile_hook() is None:
        try:
            from trn_agent_boot.trn_boot import _ntff_profile_via_ctypes

            hook = _ntff_profile_via_ctypes("/opt/axon/libaxon_pjrt.so")
            if hook is not None:
                axon_hooks.set_axon_ntff_profile_hook(hook)
        except Exception:
            pass


def kernel(pred_points: np.ndarray, gt_points: np.ndarray, _want_trace: bool = False):
    pred = np.ascontiguousarray(np.asarray(pred_points, dtype=np.float32))
    gt = np.ascontiguousarray(np.asarray(gt_points, dtype=np.float32))
    assert pred.shape == (B, N, D) and gt.shape == (B, M, D)

    in_maps = []
    for b in range(B):
        p, g = pred[b], gt[b]
        in_maps.append(
            {
                "predT": np.ascontiguousarray(p.T),
                "gtT": np.ascontiguousarray(g.T),
                "predW": np.ascontiguousarray(
                    p.reshape(NT, 128, 3).transpose(1, 0, 2).reshape(128, 3 * NT)
                ),
                "gtW": np.ascontiguousarray(
                    g.reshape(NT, 128, 3).transpose(1, 0, 2).reshape(128, 3 * NT)
                ),
            }
        )

    nc = _get_nc()
    if _want_trace:
        _ensure_ntff_hook()
    res = run_bass_kernel_spmd(nc, in_maps, core_ids=list(range(B)), trace=_want_trace)
    per_core = np.array([r["out"][0, 0] for r in res.results], dtype=np.float64)
    loss = np.float32(per_core.mean())
    if _want_trace:
        return loss, res
    return loss


# revision 15
# speedup vs baseline: 2.1967x; 1.2492x over previous
"""Chamfer loss kernel for Trainium2 (8 NeuronCores, data-parallel over batch).

Problem: B=8, N=M=4096, D=3 fp32 point clouds.
  loss = mean_b mean_n min_m ||p_bn - g_bm||^2  +  mean_b mean_m min_n ||.||^2
  (squared euclidean, clamped at 0, matching pytorch3d norm=2 semantics)

Strategy (one batch element per core):
  - Distance tiles d[n, m] = ||p||^2 + ||g||^2 - 2 p.g come from K=7 float32r
    matmuls on the PE. float32r keeps ~12 mantissa bits, so the points are
    rounded once (consistently) and the squared norms are computed FROM the
    rounded points, split into hi+lo f32r rows so the norm contribution keeps
    full fp32 precision. Row pairing (lhsT row k x rhs row k):
       k0: 1 * |g|^2_hi   k1: 1 * |g|^2_lo
       k2: |p|^2_hi * 1   k3: |p|^2_lo * 1
       k4-6: (-2 p_d) * g_d
    The resulting d error is only the point-rounding perturbation (~2e-4
    relative on coordinates), which moves the final loss by ~1e-5 relative.
  - The K=7 strips are replicated at PE row-group partition bases
    {0, 32, 64, 96} and 4 matmuls run CONCURRENTLY via tile_position row
    tiling (measured ~59us for all 256 [128, 512] tiles vs 456us for fp32).
  - Row minima: fused vector.tensor_scalar reads each 4-bank PSUM span once:
    out = relu(d) cast to bf16 into SBUF (feeds the column path), accum_out =
    running min along free, chained across the two spans per n-tile.
  - Column minima: bf16 tensor_tensor min into colacc[128, 4096], then a
    PE-transpose + free-axis reduce tail for the partition-axis min.
  - Per-core scalar output (cham_x_b + cham_y_b); the host averages the 8
    per-core scalars (the data-parallel gather).

All arithmetic happens on-chip; the host only reshapes/transposes inputs
(layout) and averages the per-core partial losses (unshard).
"""

import os
import sys

import numpy as np

sys.path.insert(0, "/opt/trn_rl_repo")

import bass_rust
import concourse.bass as bass
import concourse.mybir as mybir
from concourse.bass_utils import run_bass_kernel_spmd
from concourse.masks import make_identity
from concourse.tile import TileContext

B, N, M, D = 8, 4096, 4096, 3
NT = N // 128  # 32 n-tiles
K = 7
F32 = mybir.dt.float32
F32R = mybir.dt.float32r
BF16 = mybir.dt.bfloat16
BIG = 3.0e38

# ---------------------------------------------------------------------------
# walrus in this container rejects >1 sync-wait per instruction; spill the
# extras onto engine-matched NoOps placed immediately before the instruction.
_nop_counter = [0]


def _split_multi_waits(nc):
    for func in nc.m.functions:
        for bb in func.blocks:
            out = []
            dirty = False
            for inst in bb.instructions:
                si = inst.sync_info
                if si is not None and len(si.on_wait) > 1:
                    waits = list(si.on_wait)
                    for w in waits[:-1]:
                        _nop_counter[0] += 1
                        nop = mybir.InstNoOp(
                            name=f"I-waitsplit-{_nop_counter[0]}", ins=[], outs=[]
                        )
                        nop.engine = inst.engine
                        nop.sync_info = bass_rust.SyncInfo(on_wait=[w], on_update=[])
                        out.append(nop)
                    inst.sync_info = bass_rust.SyncInfo(
                        on_wait=[waits[-1]], on_update=list(si.on_update)
                    )
                    dirty = True
                out.append(inst)
            if dirty:
                bb.instructions = out
    return nc


# ---------------------------------------------------------------------------


def _build_prep_side(nc, tc, pool, zrows, w_dram, t_dram, ident, scale, hi_row, lo_row, c_row):
    """Build one side's K=7 rows (at partition base 0) in `zrows` [K, 4096] f32r.

    - rows c_row..c_row+2: f32r-rounded coords (scaled by `scale`), from the
      host-transposed [3, 4096] layout.
    - rows hi_row/lo_row: hi/lo split of the squared norms of the ROUNDED
      points, computed in the wide [128, 96] layout (point 128t+p at
      partition p, cols 3t..3t+2) and flattened into n-order via
      PE-transpose + a size-matched SBUF-to-SBUF DMA.
    Ones rows are pre-filled by the caller's memset.
    """
    nm = t_dram.name
    # --- coords: fp32 staging -> scale -> f32r (round; on ACT, DVE is busy) ---
    stage = pool.tile([3, N], F32, name=f"stage_{nm}")
    nc.sync.dma_start(out=stage, in_=t_dram.ap())
    coords_r = pool.tile([3, N], F32R, name=f"coords_{nm}")
    nc.scalar.mul(out=coords_r, in_=stage, mul=scale)
    nc.scalar.dma_start(out=zrows[c_row : c_row + 3, :], in_=coords_r)

    # --- norms of rounded points, wide layout ---
    w = pool.tile([128, 3 * NT], F32, name=f"w_{nm}")
    nc.sync.dma_start(out=w, in_=w_dram.ap())
    wr = pool.tile([128, 3 * NT], F32R, name=f"wr_{nm}")
    # match the coords rounding exactly: scale then round, then undo the
    # scale sign on squaring (scale^2 factored out below)
    nc.scalar.mul(out=wr, in_=w, mul=scale)
    wsq = pool.tile([128, 3 * NT], F32, name=f"wsq_{nm}")
    # (scale*p)^2 = scale^2 * p^2 -> divide back by scale^2 via tensor_scalar
    nc.vector.tensor_mul(wsq, wr, wr)
    norms = pool.tile([128, NT], F32, name=f"norms_{nm}")
    nc.vector.tensor_reduce(
        out=norms,
        in_=wsq.rearrange("p (t d) -> p t d", d=3),
        axis=mybir.AxisListType.X,
        op=mybir.AluOpType.add,
    )
    if scale != 1.0:
        nc.vector.tensor_scalar(
            out=norms,
            in0=norms,
            scalar1=1.0 / (scale * scale),
            scalar2=None,
            op0=mybir.AluOpType.mult,
        )
    # hi/lo split
    nh = pool.tile([128, NT], F32R, name=f"nh_{nm}")
    nc.vector.tensor_copy(nh, norms)
    nl_f = pool.tile([128, NT], F32, name=f"nlf_{nm}")
    nc.vector.tensor_sub(nl_f, norms, nh.bitcast(F32))
    nl = pool.tile([128, NT], F32R, name=f"nl_{nm}")
    nc.vector.tensor_copy(nl, nl_f)
    # transpose [128, 32] -> [32, 128] and flatten (c = 128 t + p) into zrows
    with tc.tile_pool(name=f"pst_{nm}", bufs=1, space="PSUM") as psp:
        for row, src in ((hi_row, nh), (lo_row, nl)):
            tn_ps = psp.tile([128, 128], F32, name=f"tnps_{nm}_{row}", tag="tnps")
            nc.tensor.matmul(
                tn_ps[0:NT, :],
                src.bitcast(F32),
                ident,
                is_transpose=True,
                start=True,
                stop=True,
            )
            tn = pool.tile([NT, 128], F32R, name=f"tn_{nm}_{row}")
            nc.scalar.copy(tn, tn_ps[0:NT, :])
            nc.sync.dma_start(out=zrows[row : row + 1, :], in_=tn)


def build_nc():
    nc = bass.Bass("TRN2")
    predT = nc.dram_tensor("predT", [3, N], F32, kind="ExternalInput")
    gtT = nc.dram_tensor("gtT", [3, M], F32, kind="ExternalInput")
    predW = nc.dram_tensor("predW", [128, 3 * NT], F32, kind="ExternalInput")
    gtW = nc.dram_tensor("gtW", [128, 3 * NT], F32, kind="ExternalInput")
    out_d = nc.dram_tensor("out", [1, 1], F32, kind="ExternalOutput")

    with TileContext(nc) as tc:
        with (
            tc.tile_pool(name="persist", bufs=1) as persist,
            tc.tile_pool(name="dsb", bufs=3) as dsbp,
        ):
            ident = persist.tile([128, 128], F32)
            make_identity(nc, ident)
            ident16 = persist.tile([128, 128], BF16)
            make_identity(nc, ident16)

            # K=7 operand strips, built directly at row-group base 0 of the
            # packed tiles, then replicated at bases 32/64/96.
            zp_pk = persist.tile([96 + K, N], F32R)
            zg_pk = persist.tile([96 + K, M], F32R)
            zp = zp_pk[0:K, :]
            zg = zg_pk[0:K, :]
            # ones rows: Memset can't target f32r, and compute ops can't
            # start at unaligned partitions — build a rounded ones strip at
            # partition 0 and DMA it into place.
            ones_f = persist.tile([2, N], F32)
            nc.gpsimd.memset(ones_f, 1.0)
            ones_r = persist.tile([2, N], F32R)
            nc.scalar.copy(ones_r, ones_f)
            nc.sync.dma_start(out=zp[0:2, :], in_=ones_r)
            nc.scalar.dma_start(out=zg[2:4, :], in_=ones_r)
            # zp rows: [1, 1, nPh, nPl, -2p0, -2p1, -2p2]
            # zg rows: [nGh, nGl, 1, 1, g0, g1, g2]
            _build_prep_side(nc, tc, persist, zp, predW, predT, ident, -2.0, 2, 3, 4)
            _build_prep_side(nc, tc, persist, zg, gtW, gtT, ident, 1.0, 0, 1, 4)

            # replicate both strips at PE row-group bases 32/64/96,
            # split across the two HWDGE queues
            for g in range(1, 4):
                eng = nc.sync if g % 2 else nc.scalar
                eng.dma_start(out=zp_pk[32 * g : 32 * g + K, :], in_=zp)
                eng2 = nc.scalar if g % 2 else nc.sync
                eng2.dma_start(out=zg_pk[32 * g : 32 * g + K, :], in_=zg)

            rowmins = persist.tile([128, NT], F32)
            colacc = persist.tile([128, M], BF16)

            # ---- main loop: 32 n-tiles x 2 half-spans of [128, 2048];
            #      each span = 4 row-group-packed concurrent matmuls ----
            # Pipeline per n-tile: PE (4 packed MMs per half-span) -> ACT
            # copies PSUM->SBUF bf16 -> DVE rowmin fold-tree (bf16 2x mode)
            # + colmin accumulation. The DVE never touches PSUM, so its
            # per-tile cost drops from ~6.4us (1x PSUM reduce) to ~4.8us.
            # Clamping at 0 happens once on the tiny minima tiles at the end.
            with tc.tile_pool(name="psum_main", bufs=2, space="PSUM") as psp:
                for t in range(NT):
                    dsb = (
                        colacc
                        if t == 0
                        else dsbp.tile([128, M], BF16, name="dsb", tag="dsb")
                    )
                    for h in range(2):
                        ps = psp.tile([128, 2048], F32, name="ps_main", tag="ps_main")
                        for g in range(4):
                            col0 = 2048 * h + 512 * g
                            nc.tensor.matmul(
                                ps[:, 512 * g : 512 * (g + 1)],
                                zp_pk[32 * g : 32 * g + K, 128 * t : 128 * (t + 1)],
                                zg_pk[32 * g : 32 * g + K, col0 : col0 + 512],
                                start=True,
                                stop=True,
                                tile_position=(32 * g, 0),
                            )
                        nc.scalar.copy(
                            out=dsb[:, 2048 * h : 2048 * (h + 1)], in_=ps
                        )
                    # rowmin: bf16 fold-tree (each TT runs in DVE 2x mode)
                    f1 = dsbp.tile([128, 2048], BF16, name="f1", tag="f1")
                    nc.vector.tensor_tensor(
                        out=f1, in0=dsb[:, 0:2048], in1=dsb[:, 2048:4096],
                        op=mybir.AluOpType.min,
                    )
                    f2 = dsbp.tile([128, 1024], BF16, name="f2", tag="f2")
                    nc.vector.tensor_tensor(
                        out=f2, in0=f1[:, 0:1024], in1=f1[:, 1024:2048],
                        op=mybir.AluOpType.min,
                    )
                    f3 = dsbp.tile([128, 512], BF16, name="f3", tag="f3")
                    nc.vector.tensor_tensor(
                        out=f3, in0=f2[:, 0:512], in1=f2[:, 512:1024],
                        op=mybir.AluOpType.min,
                    )
                    nc.vector.tensor_reduce(
                        out=rowmins[:, t : t + 1],
                        in_=f3,
                        axis=mybir.AxisListType.X,
                        op=mybir.AluOpType.min,
                    )
                    if t > 0:
                        nc.vector.tensor_tensor(
                            out=colacc,
                            in0=dsb,
                            in1=colacc,
                            op=mybir.AluOpType.min,
                        )

            # ---- tail: min over partitions of colacc via PE transpose ----
            colmins = persist.tile([128, NT], F32)
            with tc.tile_pool(name="psum_tail", bufs=2, space="PSUM") as psp2:
                for g in range(8):  # 8 groups of 4 [128,128] blocks
                    pst = psp2.tile([128, 512], BF16, name="ps_tr", tag="ps_tr")
                    for k in range(4):
                        b = 4 * g + k
                        nc.tensor.matmul(
                            pst[:, 128 * k : 128 * (k + 1)],
                            colacc[:, 128 * b : 128 * (b + 1)],
                            ident16,
                            is_transpose=True,
                            start=True,
                            stop=True,
                        )
                    nc.vector.tensor_reduce(
                        out=colmins[:, 4 * g : 4 * (g + 1)],
                        in_=pst.rearrange("p (k i) -> p k i", i=128),
                        axis=mybir.AxisListType.X,
                        op=mybir.AluOpType.min,
                    )

                # ---- final sums (clamp the minima at 0 first) ----
                for mins in (rowmins, colmins):
                    nc.vector.tensor_scalar(
                        out=mins, in0=mins, scalar1=0.0, scalar2=None,
                        op0=mybir.AluOpType.max,
                    )
                rowsum = persist.tile([128, 1], F32)
                colsum = persist.tile([128, 1], F32)
                nc.vector.tensor_reduce(
                    out=rowsum,
                    in_=rowmins,
                    axis=mybir.AxisListType.X,
                    op=mybir.AluOpType.add,
                )
                nc.vector.tensor_reduce(
                    out=colsum,
                    in_=colmins,
                    axis=mybir.AxisListType.X,
                    op=mybir.AluOpType.add,
                )
                total = persist.tile([128, 1], F32)
                nc.vector.tensor_add(total, rowsum, colsum)
                ps_s = psp2.tile([1, 1], F32, name="ps_s", tag="ps_s")
                ones = nc.const_aps.tensor(1.0, (128, 1))
                nc.tensor.matmul(ps_s, ones, total, start=True, stop=True)
                res_sb = persist.tile([1, 1], F32)
                nc.scalar.mul(res_sb, ps_s, 1.0 / float(N))
                nc.sync.dma_start(out=out_d.ap(), in_=res_sb)

    _split_multi_waits(nc)
    return nc


_NC = None


def _get_nc():
    global _NC
    if _NC is None:
        _NC = build_nc()
    return _NC


def _ensure_ntff_hook():
    """Register the axon NTFF profiling hook if the container's antenv stub
    lacks axon_hooks (trace support; harmless to skip)."""
    import types

    try:
        import antenv
    except ImportError:
        return
    if not hasattr(antenv, "axon_hooks") or not hasattr(
        getattr(antenv, "axon_hooks", None), "get_axon_ntff_profile_hook"
    ):
        mod = types.ModuleType("antenv.axon_hooks")
        mod._h = None
        mod.set_axon_ntff_profile_hook = lambda h: setattr(mod, "_h", h)
        mod.get_axon_ntff_profile_hook = lambda: mod._h
        sys.modules["antenv.axon_hooks"] = mod
        antenv.axon_hooks = mod
    from antenv import axon_hooks

    if axon_hooks.get_axon_ntff_profile_hook() is None:
        try:
            from trn_agent_boot.trn_boot import _ntff_profile_via_ctypes

            hook = _ntff_profile_via_ctypes("/opt/axon/libaxon_pjrt.so")
            if hook is not None:
                axon_hooks.set_axon_ntff_profile_hook(hook)
        except Exception:
            pass


def kernel(pred_points: np.ndarray, gt_points: np.ndarray, _want_trace: bool = False):
    pred = np.ascontiguousarray(np.asarray(pred_points, dtype=np.float32))
    gt = np.ascontiguousarray(np.asarray(gt_points, dtype=np.float32))
    assert pred.shape == (B, N, D) and gt.shape == (B, M, D)

    in_maps = []
    for b in range(B):
        p, g = pred[b], gt[b]
        in_maps.append(
            {
                "predT": np.ascontiguousarray(p.T),
                "gtT": np.ascontiguousarray(g.T),
                "predW": np.ascontiguousarray(
                    p.reshape(NT, 128, 3).transpose(1, 0, 2).reshape(128, 3 * NT)
                ),
                "gtW": np.ascontiguousarray(
                    g.reshape(NT, 128, 3).transpose(1, 0, 2).reshape(128, 3 * NT)
                ),
            }
        )

    nc = _get_nc()
    if _want_trace:
        _ensure_ntff_hook()
    res = run_bass_kernel_spmd(nc, in_maps, core_ids=list(range(B)), trace=_want_trace)
    per_core = np.array([r["out"][0, 0] for r in res.results], dtype=np.float64)
    loss = np.float32(per_core.mean())
    if _want_trace:
        return loss, res
    return loss


# revision 23
# speedup vs baseline: 2.2106x; 1.0063x over previous
"""Chamfer loss kernel for Trainium2 (8 NeuronCores, data-parallel over batch).

Problem: B=8, N=M=4096, D=3 fp32 point clouds.
  loss = mean_b mean_n min_m ||p_bn - g_bm||^2  +  mean_b mean_m min_n ||.||^2
  (squared euclidean, clamped at 0, matching pytorch3d norm=2 semantics)

Strategy (one batch element per core):
  - Distance tiles d[n, m] = ||p||^2 + ||g||^2 - 2 p.g come from K=7 float32r
    matmuls on the PE. float32r keeps ~12 mantissa bits, so the points are
    rounded once (consistently) and the squared norms are computed FROM the
    rounded points, split into hi+lo f32r rows so the norm contribution keeps
    full fp32 precision. Row pairing (lhsT row k x rhs row k):
       k0: 1 * |g|^2_hi   k1: 1 * |g|^2_lo
       k2: |p|^2_hi * 1   k3: |p|^2_lo * 1
       k4-6: (-2 p_d) * g_d
    The resulting d error is only the point-rounding perturbation (~2e-4
    relative on coordinates), which moves the final loss by ~1e-5 relative.
  - The K=7 strips are replicated at PE row-group partition bases
    {0, 32, 64, 96} and 4 matmuls run CONCURRENTLY via tile_position row
    tiling (measured ~59us for all 256 [128, 512] tiles vs 456us for fp32).
  - Row minima: fused vector.tensor_scalar reads each 4-bank PSUM span once:
    out = relu(d) cast to bf16 into SBUF (feeds the column path), accum_out =
    running min along free, chained across the two spans per n-tile.
  - Column minima: bf16 tensor_tensor min into colacc[128, 4096], then a
    PE-transpose + free-axis reduce tail for the partition-axis min.
  - Per-core scalar output (cham_x_b + cham_y_b); the host averages the 8
    per-core scalars (the data-parallel gather).

All arithmetic happens on-chip; the host only reshapes/transposes inputs
(layout) and averages the per-core partial losses (unshard).
"""

import os
import sys

import numpy as np

sys.path.insert(0, "/opt/trn_rl_repo")

import bass_rust
import concourse.bass as bass
import concourse.mybir as mybir
from concourse.bass_utils import run_bass_kernel_spmd
from concourse.masks import make_identity
from concourse.tile import TileContext

B, N, M, D = 8, 4096, 4096, 3
NT = N // 128  # 32 n-tiles
K = 7
F32 = mybir.dt.float32
F32R = mybir.dt.float32r
BF16 = mybir.dt.bfloat16
BIG = 3.0e38

# ---------------------------------------------------------------------------
# walrus in this container rejects >1 sync-wait per instruction; spill the
# extras onto engine-matched NoOps placed immediately before the instruction.
_nop_counter = [0]


def _split_multi_waits(nc):
    for func in nc.m.functions:
        for bb in func.blocks:
            out = []
            dirty = False
            for inst in bb.instructions:
                si = inst.sync_info
                if si is not None and len(si.on_wait) > 1:
                    waits = list(si.on_wait)
                    for w in waits[:-1]:
                        _nop_counter[0] += 1
                        nop = mybir.InstNoOp(
                            name=f"I-waitsplit-{_nop_counter[0]}", ins=[], outs=[]
                        )
                        nop.engine = inst.engine
                        nop.sync_info = bass_rust.SyncInfo(on_wait=[w], on_update=[])
                        out.append(nop)
                    inst.sync_info = bass_rust.SyncInfo(
                        on_wait=[waits[-1]], on_update=list(si.on_update)
                    )
                    dirty = True
                out.append(inst)
            if dirty:
                bb.instructions = out
    return nc


# ---------------------------------------------------------------------------


def _rep4(engines, zpk, row, nrows, src):
    """Replicate `src` into rows [row, row+nrows) of each of the 4 PE
    row-group copies in zpk, alternating DMA queues."""
    for g in range(4):
        engines[g % len(engines)].dma_start(
            out=zpk[32 * g + row : 32 * g + row + nrows, :], in_=src
        )


def _build_prep_side(nc, tc, pool, zpk, w_dram, ident, scale, hi_row, lo_row, c_row, dma_eng):
    """Build one side's K=7 rows in all 4 row groups of zpk [96+K, 4096] f32r.

    Everything derives from the wide input layout [128, 96] (point 128t+p at
    partition p, cols 3t..3t+2): rounded coords via PE-transpose, squared
    norms (of the rounded points) hi/lo-split, both flattened into n-order
    and replicated to the 4 PE row-group copies by single broadcast DMAs.
    """
    nm = w_dram.name
    w = pool.tile([128, 3 * NT], F32, name=f"w_{nm}")
    nc.sync.dma_start(out=w, in_=w_dram.ap())
    # round (and scale) once; coords and norms both come from wr
    wr = pool.tile([128, 3 * NT], F32R, name=f"wr_{nm}")
    nc.scalar.mul(out=wr, in_=w, mul=scale)

    with tc.tile_pool(name=f"pst_{nm}", bufs=1, space="PSUM") as psp:
        # --- coords: transpose wr -> [96, 128], flatten+replicate per dim ---
        tw_ps = psp.tile([128, 128], F32, name=f"twps_{nm}", tag="twps")
        nc.tensor.matmul(
            tw_ps[0 : 3 * NT, :],
            wr.bitcast(F32),
            ident,
            is_transpose=True,
            start=True,
            stop=True,
        )
        tw = pool.tile([3 * NT, 128], F32R, name=f"tw_{nm}")
        nc.scalar.copy(tw, tw_ps[0 : 3 * NT, :])
        tw_d = tw.rearrange("(t d) p -> d t p", d=3)
        for d in range(3):
            _rep4(dma_eng, zpk, c_row + d, 1, tw_d[d])

        # --- norms of rounded points ---
        wsq = pool.tile([128, 3 * NT], F32, name=f"wsq_{nm}")
        nc.vector.tensor_mul(wsq, wr, wr)
        norms = pool.tile([128, NT], F32, name=f"norms_{nm}")
        nc.vector.tensor_reduce(
            out=norms,
            in_=wsq.rearrange("p (t d) -> p t d", d=3),
            axis=mybir.AxisListType.X,
            op=mybir.AluOpType.add,
        )
        if scale != 1.0:
            # norms of scale*p -> divide by scale^2 (exact for powers of 2)
            nc.vector.tensor_scalar(
                out=norms,
                in0=norms,
                scalar1=1.0 / (scale * scale),
                scalar2=None,
                op0=mybir.AluOpType.mult,
            )
        nh = pool.tile([128, NT], F32R, name=f"nh_{nm}")
        nc.vector.tensor_copy(nh, norms)
        nl_f = pool.tile([128, NT], F32, name=f"nlf_{nm}")
        nc.vector.tensor_sub(nl_f, norms, nh.bitcast(F32))
        nl = pool.tile([128, NT], F32R, name=f"nl_{nm}")
        nc.vector.tensor_copy(nl, nl_f)
        for row, src in ((hi_row, nh), (lo_row, nl)):
            tn_ps = psp.tile([128, 128], F32, name=f"tnps_{nm}_{row}", tag="tnps")
            nc.tensor.matmul(
                tn_ps[0:NT, :],
                src.bitcast(F32),
                ident,
                is_transpose=True,
                start=True,
                stop=True,
            )
            tn = pool.tile([NT, 128], F32R, name=f"tn_{nm}_{row}")
            nc.scalar.copy(tn, tn_ps[0:NT, :])
            _rep4(dma_eng, zpk, row, 1, tn)


def build_nc():
    nc = bass.Bass("TRN2")
    predW = nc.dram_tensor("predW", [128, 3 * NT], F32, kind="ExternalInput")
    gtW = nc.dram_tensor("gtW", [128, 3 * NT], F32, kind="ExternalInput")
    out_d = nc.dram_tensor("out", [1, 1], F32, kind="ExternalOutput")

    with TileContext(nc) as tc:
        with (
            tc.tile_pool(name="persist", bufs=1) as persist,
            tc.tile_pool(name="dsb", bufs=3) as dsbp,
        ):
            ident = persist.tile([128, 128], F32)
            make_identity(nc, ident)
            ident16 = persist.tile([128, 128], BF16)
            make_identity(nc, ident16)

            # K=7 operand strips, all 4 PE row-group copies written directly
            # by broadcast DMAs.
            zp_pk = persist.tile([96 + K, N], F32R)
            zg_pk = persist.tile([96 + K, M], F32R)
            # ones rows (Memset can't target f32r; compute ops can't start at
            # unaligned partitions): rounded ones strip -> broadcast DMA.
            ones_f = persist.tile([2, N], F32)
            nc.gpsimd.memset(ones_f, 1.0)
            ones_r = persist.tile([2, N], F32R)
            nc.scalar.copy(ones_r, ones_f)
            _rep4([nc.sync, nc.scalar], zp_pk, 0, 2, ones_r)
            _rep4([nc.scalar, nc.sync], zg_pk, 2, 2, ones_r)
            # zp rows: [1, 1, nPh, nPl, -2p0, -2p1, -2p2]
            # zg rows: [nGh, nGl, 1, 1, g0, g1, g2]
            _build_prep_side(
                nc, tc, persist, zp_pk, predW, ident, -2.0, 2, 3, 4, [nc.sync, nc.scalar]
            )
            _build_prep_side(
                nc, tc, persist, zg_pk, gtW, ident, 1.0, 0, 1, 4, [nc.scalar, nc.sync]
            )

            rowmins = persist.tile([128, NT], F32)
            colacc = persist.tile([128, M], BF16)

            # ---- main loop: 32 n-tiles x 2 half-spans of [128, 2048];
            #      each span = 4 row-group-packed concurrent matmuls ----
            # Pipeline per n-tile: PE (4 packed MMs per half-span) -> ACT
            # copies PSUM->SBUF bf16 -> DVE rowmin fold-tree (bf16 2x mode)
            # + colmin accumulation. The DVE never touches PSUM, so its
            # per-tile cost drops from ~6.4us (1x PSUM reduce) to ~4.8us.
            # Clamping at 0 happens once on the tiny minima tiles at the end.
            with tc.tile_pool(name="psum_main", bufs=2, space="PSUM") as psp:
                for t in range(NT):
                    dsb = (
                        colacc
                        if t == 0
                        else dsbp.tile([128, M], BF16, name="dsb", tag="dsb")
                    )
                    for h in range(2):
                        ps = psp.tile([128, 2048], F32, name="ps_main", tag="ps_main")
                        for g in range(4):
                            col0 = 2048 * h + 512 * g
                            nc.tensor.matmul(
                                ps[:, 512 * g : 512 * (g + 1)],
                                zp_pk[32 * g : 32 * g + K, 128 * t : 128 * (t + 1)],
                                zg_pk[32 * g : 32 * g + K, col0 : col0 + 512],
                                start=True,
                                stop=True,
                                tile_position=(32 * g, 0),
                            )
                        nc.scalar.copy(
                            out=dsb[:, 2048 * h : 2048 * (h + 1)], in_=ps
                        )
                    # rowmin: bf16 fold-tree (each TT runs in DVE 2x mode)
                    f1 = dsbp.tile([128, 2048], BF16, name="f1", tag="f1")
                    nc.vector.tensor_tensor(
                        out=f1, in0=dsb[:, 0:2048], in1=dsb[:, 2048:4096],
                        op=mybir.AluOpType.min,
                    )
                    f2 = dsbp.tile([128, 1024], BF16, name="f2", tag="f2")
                    nc.vector.tensor_tensor(
                        out=f2, in0=f1[:, 0:1024], in1=f1[:, 1024:2048],
                        op=mybir.AluOpType.min,
                    )
                    f3 = dsbp.tile([128, 512], BF16, name="f3", tag="f3")
                    nc.vector.tensor_tensor(
                        out=f3, in0=f2[:, 0:512], in1=f2[:, 512:1024],
                        op=mybir.AluOpType.min,
                    )
                    nc.vector.tensor_reduce(
                        out=rowmins[:, t : t + 1],
                        in_=f3,
                        axis=mybir.AxisListType.X,
                        op=mybir.AluOpType.min,
                    )
                    if t > 0:
                        nc.vector.tensor_tensor(
                            out=colacc,
                            in0=dsb,
                            in1=colacc,
                            op=mybir.AluOpType.min,
                        )

            # ---- tail: min over partitions of colacc via PE transpose ----
            colmins = persist.tile([128, NT], F32)
            with tc.tile_pool(name="psum_tail", bufs=2, space="PSUM") as psp2:
                for g in range(8):  # 8 groups of 4 [128,128] blocks
                    pst = psp2.tile([128, 512], BF16, name="ps_tr", tag="ps_tr")
                    for k in range(4):
                        b = 4 * g + k
                        nc.tensor.matmul(
                            pst[:, 128 * k : 128 * (k + 1)],
                            colacc[:, 128 * b : 128 * (b + 1)],
                            ident16,
                            is_transpose=True,
                            start=True,
                            stop=True,
                        )
                    nc.vector.tensor_reduce(
                        out=colmins[:, 4 * g : 4 * (g + 1)],
                        in_=pst.rearrange("p (k i) -> p k i", i=128),
                        axis=mybir.AxisListType.X,
                        op=mybir.AluOpType.min,
                    )

                # ---- final sums (clamp the minima at 0 first) ----
                for mins in (rowmins, colmins):
                    nc.vector.tensor_scalar(
                        out=mins, in0=mins, scalar1=0.0, scalar2=None,
                        op0=mybir.AluOpType.max,
                    )
                rowsum = persist.tile([128, 1], F32)
                colsum = persist.tile([128, 1], F32)
                nc.vector.tensor_reduce(
                    out=rowsum,
                    in_=rowmins,
                    axis=mybir.AxisListType.X,
                    op=mybir.AluOpType.add,
                )
                nc.vector.tensor_reduce(
                    out=colsum,
                    in_=colmins,
                    axis=mybir.AxisListType.X,
                    op=mybir.AluOpType.add,
                )
                total = persist.tile([128, 1], F32)
                nc.vector.tensor_add(total, rowsum, colsum)
                ps_s = psp2.tile([1, 1], F32, name="ps_s", tag="ps_s")
                ones = nc.const_aps.tensor(1.0, (128, 1))
                nc.tensor.matmul(ps_s, ones, total, start=True, stop=True)
                res_sb = persist.tile([1, 1], F32)
                nc.scalar.mul(res_sb, ps_s, 1.0 / float(N))
                nc.sync.dma_start(out=out_d.ap(), in_=res_sb)

    _split_multi_waits(nc)
    return nc


_NC = None


def _get_nc():
    global _NC
    if _NC is None:
        _NC = build_nc()
    return _NC


def _ensure_ntff_hook():
    """Register the axon NTFF profiling hook if the container's antenv stub
    lacks axon_hooks (trace support; harmless to skip)."""
    import types

    try:
        import antenv
    except ImportError:
        return
    if not hasattr(antenv, "axon_hooks") or not hasattr(
        getattr(antenv, "axon_hooks", None), "get_axon_ntff_profile_hook"
    ):
        mod = types.ModuleType("antenv.axon_hooks")
        mod._h = None
        mod.set_axon_ntff_profile_hook = lambda h: setattr(mod, "_h", h)
        mod.get_axon_ntff_profile_hook = lambda: mod._h
        sys.modules["antenv.axon_hooks"] = mod
        antenv.axon_hooks = mod
    from antenv import axon_hooks

    if axon_hooks.get_axon_ntff_profile_hook() is None:
        try:
            from trn_agent_boot.trn_boot import _ntff_profile_via_ctypes

            hook = _ntff_profile_via_ctypes("/opt/axon/libaxon_pjrt.so")
            if hook is not None:
                axon_hooks.set_axon_ntff_profile_hook(hook)
        except Exception:
            pass


def kernel(pred_points: np.ndarray, gt_points: np.ndarray, _want_trace: bool = False):
    pred = np.ascontiguousarray(np.asarray(pred_points, dtype=np.float32))
    gt = np.ascontiguousarray(np.asarray(gt_points, dtype=np.float32))
    assert pred.shape == (B, N, D) and gt.shape == (B, M, D)

    in_maps = []
    for b in range(B):
        p, g = pred[b], gt[b]
        in_maps.append(
            {
                "predW": np.ascontiguousarray(
                    p.reshape(NT, 128, 3).transpose(1, 0, 2).reshape(128, 3 * NT)
                ),
                "gtW": np.ascontiguousarray(
                    g.reshape(NT, 128, 3).transpose(1, 0, 2).reshape(128, 3 * NT)
                ),
            }
        )

    nc = _get_nc()
    if _want_trace:
        _ensure_ntff_hook()
    res = run_bass_kernel_spmd(nc, in_maps, core_ids=list(range(B)), trace=_want_trace)
    per_core = np.array([r["out"][0, 0] for r in res.results], dtype=np.float64)
    loss = np.float32(per_core.mean())
    if _want_trace:
        return loss, res
    return loss
